# revision 9
# baseline (speedup 1.0000x reference)
"""Trainium2 Bass kernel for the AttentionRNNModel problem.

Math (fp32 reference):
    xi  = x @ W_i2h.T + b_i2h                      # [B,T,H]
    h_t = tanh(xi_t + h_{t-1} @ W_h2h.T + b_h2h)   # 512 sequential steps
    out = concat_t(h_t) @ W_fc.T + b_fc            # [B, O]

Strategy: data-parallel over batch across 8 cores (16 rows each). On each
core the hidden state is kept transposed, h_T [H=8x128 partitions, 16], so
each step is 8 m-slabs x 8 k-tiles of W_h2h.T as stationary [128,128] bf16
weights with h_T k-tiles moving (N=16), accumulating in 4 PSUM bank groups
(2 slabs each). The input projection + fused bias (b_i2h + b_h2h) enter
each slab's PSUM group as one extra K=65 matmul (x_t augmented with a ones
row); all 8 of these are emitted first each step — they don't read h, so
they give PE runway covering the previous step's last tanh + semaphore
latency. tanh fires per bank-group as soon as its slabs finish (ScalarE,
PSUM->SBUF bf16), producing the next step's h_T directly in matmul layout —
no transposes anywhere. The final FC is folded in as 8 matmuls per step
accumulating W_fc_t.T @ h_t into a PSUM accumulator left open across all
512 steps; only [24,16] per core is DMA'd out at the end. W_fc streams in
per-step (padded 24->128 weight columns: FWL needs exactly 128 columns,
and non-FWL pairs cost ~70ns vs ~34ns). Everything else is SBUF-resident.

Measured on the 8-core axon TRN2: 1.36 ms HW exec, rel err 3.1e-3.
The kernel is purely PE-instruction-bound: 80 LDWEIGHTS+MATMUL pairs/step
x ~34 ns/pair (the hardware floor for full-weight-reload bf16 pairs at
N=16); all DMA/tanh/semaphore latency is hidden behind the matmul stream.
"""

import numpy as np
import ml_dtypes

import concourse.bass as bass
import concourse.tile as tile
from concourse import bacc, mybir
from concourse.bass_utils import run_bass_kernel_spmd

B, T, D, H, O = 128, 512, 64, 1024, 24
NCORES = 8
BC = B // NCORES          # batch per core = 16
KM = H // 128             # 8 k-tiles / m-slabs
BF16 = mybir.dt.bfloat16
F8E3 = mybir.dt.float8e3
F32 = mybir.dt.float32
bf16 = ml_dtypes.bfloat16
f8e3 = ml_dtypes.float8_e3m4

# W_h2h stationary weights in fp8 e3m4: FWL loads 4 elems per 32-bit read
# (vs 2 for bf16), halving the LDWEIGHTS-bound pair cost of the 64 W
# matmuls per step. Values pre-scaled x256 so all of U(-1/32,1/32) lands in
# e3m4 normal range (max 15.5, min normal 0.25); the inverse scale rides
# the tanh activation (out = tanh(psum * 1/256)), so wiT (input proj +
# fused bias, still bf16) is pre-scaled x256 too to match the PSUM scale.
W_FP8 = True
W_SCALE = 256.0 if W_FP8 else 1.0
INV_W_SCALE = 1.0 / W_SCALE


def _build_program(t_steps: int, reps: int = 1, act_groups: int = None,
                   ps_bufs: int = 1):
    global ACT_GROUPS
    if act_groups is not None:
        ACT_GROUPS = act_groups
    nc = bacc.Bacc("TRN2", target_bir_lowering=False, debug=False)

    wT_d = nc.dram_tensor("wT", [128, KM, H], F8E3 if W_FP8 else BF16,
                          kind="ExternalInput")
    wiT_d = nc.dram_tensor("wiT", [D + 1, KM, 128], BF16, kind="ExternalInput")
    xTa_d = nc.dram_tensor("xTa", [D + 1, t_steps, BC], BF16, kind="ExternalInput")
    fcw = O if FC_FLIP else FC_PAD
    wfc_d = nc.dram_tensor("wfc", [t_steps, 128, KM, fcw], BF16, kind="ExternalInput")
    out_shape = [BC, O] if FC_FLIP else [O, BC]
    out_d = nc.dram_tensor("out", out_shape, F32, kind="ExternalOutput")

    with tile.TileContext(nc) as tc:
        with (
            tc.tile_pool(name="const", bufs=1) as const_pool,
            tc.tile_pool(name="wfc", bufs=WFC_BUFS) as wfc_pool,
            tc.tile_pool(name="h", bufs=H_BUFS) as h_pool,
            tc.tile_pool(name="ps", bufs=ps_bufs, space=bass.MemorySpace.PSUM) as ps_pool,
            tc.tile_pool(name="fcps", bufs=1, space=bass.MemorySpace.PSUM) as fcps_pool,
            tc.tile_pool(name="outp", bufs=1) as out_pool,
        ):
            if CONST_IN_LOOP == 0:
                wT, wiT, xTa = _emit_const_loads(
                    nc, const_pool, t_steps, wT_d, wiT_d, xTa_d)

            fc_ps = fcps_pool.tile([BC, O] if FC_FLIP else [FC_PAD, BC], F32)

            import contextlib
            rep_ctx = tc.For_i(0, reps) if reps > 1 else contextlib.nullcontext()
            with rep_ctx:
                if CONST_IN_LOOP:
                    wT, wiT, xTa = _emit_const_loads(
                        nc, const_pool, t_steps, wT_d, wiT_d, xTa_d,
                        mono=(CONST_IN_LOOP == "mono"))
                _emit_body(
                    nc, tc, t_steps, wT, wiT, xTa, fc_ps,
                    wfc_pool, h_pool, ps_pool, out_pool, wfc_d, out_d,
                )

    nc.compile()
    return nc


WFC_BUFS = 4  # 4-24 all equal within measurement noise
H_BUFS = 3
# 0 = const loads before the rep loop (production); "split"/"mono" put them
# inside the loop so the reps-slope measures the startup stall (dev only)
CONST_IN_LOOP = 0


def _emit_const_loads(nc, const_pool, t_steps, wT_d, wiT_d, xTa_d, mono=False):
    # Split into chunks across two DMA engines so the first steps start
    # ~5us in instead of waiting for monolithic transfers (matters for the
    # one-shot run; the steady-state rep loop never sees it).
    wiT = const_pool.tile([D + 1, KM, 128], BF16, tag="wiT", name="wiT")
    nc.sync.dma_start(wiT[:], wiT_d[:])
    xTa = const_pool.tile([D + 1, t_steps, BC], BF16, tag="xTa", name="xTa")
    wT = const_pool.tile([128, KM, H], F8E3 if W_FP8 else BF16, tag="wT",
                         name="wT")
    if mono:
        nc.gpsimd.dma_start(xTa[:], xTa_d[:])
        nc.sync.dma_start(wT[:], wT_d[:])
        return wT, wiT, xTa
    nxc = 4
    xc_span = t_steps // nxc if t_steps % nxc == 0 else t_steps
    nxc = t_steps // xc_span
    nc.gpsimd.dma_start(xTa[:, 0:xc_span, :], xTa_d[:, 0:xc_span, :])
    for k in range(KM):
        eng = nc.sync if k % 2 == 0 else nc.gpsimd
        eng.dma_start(wT[:, k, :], wT_d[:, k, :])
    for c in range(1, nxc):
        nc.gpsimd.dma_start(
            xTa[:, c * xc_span:(c + 1) * xc_span, :],
            xTa_d[:, c * xc_span:(c + 1) * xc_span, :],
        )
    return wT, wiT, xTa
ACT_GROUPS = 4  # m-slabs per tanh chunk = KM // ACT_GROUPS; one PSUM bank per group

# Ablation flags (timing experiments only — wrong results when set)
ABL = dict(no_fc=False, no_xb=False, no_w=False, no_act=False, static_wfc=False,
           free_run=False)
# FC matmul placement: "tail" = 8 consecutive after the W MMs,
# "interleave" = one after each m-slab (spreads same-address PSUM accums)
FC_MODE = "tail"
# Pad FC weights 24 -> FC_PAD columns so the weight load is FWL-eligible
# (requires 128 weight columns); extra PSUM rows are wasted but free.
# Only relevant when FC_FLIP is False.
FC_PAD = 128
# FC_FLIP: use h as the stationary operand (16-col LDWEIGHTS) and wfc as
# the moving operand (24 cols, no padding needed) -> PSUM accumulator is
# [16, 24] batch-major and wfc stays compact (48KB/step vs 256KB).
FC_FLIP = False  # measured worse: non-FWL (small-col) weight loads have a
# ~70ns pair floor regardless of LDW size; padded-FWL wfc wins
# NOTE: an xi-precompute variant (phase-0 GEMM + per-chunk DVE add instead
# of the 8 per-step xb matmuls) was measured and removed: the DVE adds on
# PSUM cost about what the saved matmuls gain.


def _emit_body(nc, tc, t_steps, wT, wiT, xTa, fc_ps,
               wfc_pool, h_pool, ps_pool, out_pool, wfc_d, out_d):
    # PSUM split into ACT_GROUPS tiles (one bank each, bufs=1, reused every
    # step; the WAR on each group's tanh gates reuse). tanh fires per group
    # as soon as its slabs' matmuls finish, so activations ride just behind
    # the matmul wavefront and the step-boundary bubble is only the last
    # group's tanh, itself overlapped with the FC matmuls of h_{t-1}.
    GS = KM // ACT_GROUPS
    h_prev = None
    wfc_prev = None
    if ABL["free_run"]:
        # diagnostic: 64 W MMs/step against a constant rhs, no deps at all.
        # free_run = "m" (m-major), "k" (k-major: consecutive MMs rotate
        # across all 8 psum regions), "m2" (pairs of slabs alternate)
        order = ABL["free_run"]
        h_const = h_pool.tile([128, KM, BC], BF16, tag="hconst", name="hconst")
        nc.vector.memset(h_const[:], 0.0)
        for t in range(t_steps):
            pss = [
                ps_pool.tile([128, GS, BC], F32, tag=f"psg{g}", name=f"psf{g}")
                for g in range(ACT_GROUPS)
            ]
            if order == "k":
                mks = [(m, k) for k in range(KM) for m in range(KM)]
            elif order == "m2":
                mks = [(m0 + i, k) for m0 in range(0, KM, 2)
                       for k in range(KM) for i in range(2)]
            else:
                mks = [(m, k) for m in range(KM) for k in range(KM)]
            seen = set()
            for m, k in mks:
                g, sl = divmod(m, GS)
                last = len(seen) == 63
                nc.tensor.matmul(
                    pss[g][:, sl, :],
                    wT[:, k, m * 128:(m + 1) * 128],
                    h_const[:, k, :],
                    start=(m, k) == mks[0],
                    stop=last,
                    skip_group_check=True,
                )
                seen.add((m, k))
        out_sb = out_pool.tile([O, BC], F32, name="outsb")
        nc.vector.memset(out_sb[:], 0.0)
        nc.sync.dma_start(out_d[:], out_sb[:])
        return
    if ABL["static_wfc"]:
        wfc_static = wfc_pool.tile([128, KM, O if FC_FLIP else FC_PAD], BF16,
                                    tag="wfcstatic")
        nc.sync.dma_start(wfc_static[:], wfc_d[0])
    for t in range(t_steps):
        if ABL["static_wfc"]:
            wfc_t = wfc_static
        else:
            wfc_t = wfc_pool.tile([128, KM, O if FC_FLIP else FC_PAD], BF16)
            nc.sync.dma_start(wfc_t[:], wfc_d[t])

        h_new = h_pool.tile([128, KM, BC], BF16)
        ps_g = []
        # Phase 1 — all xb matmuls (input projection + bias). These read
        # only xTa, not h, so they give PE ~8 MMs of runway at each step
        # boundary that covers the previous step's last tanh + sem latency.
        have_w = t > 0 and not ABL["no_w"]
        for m in range(KM):
            g, sl = divmod(m, GS)
            if sl == 0:
                ps_g.append(ps_pool.tile([128, GS, BC], F32, tag=f"psg{g}",
                                         name=f"ps{g}"))
            if not ABL["no_xb"] or sl == 0:
                nc.tensor.matmul(
                    ps_g[g][:, sl, :],
                    wiT[:, m, :],
                    xTa[:, t, :],
                    start=(sl == 0),
                    stop=(sl == GS - 1 and not have_w),
                )
            if not have_w and sl == GS - 1 and not ABL["no_act"]:
                nc.scalar.activation(
                    h_new[:, g * GS:(g + 1) * GS, :], ps_g[g][:],
                    mybir.ActivationFunctionType.Tanh, scale=INV_W_SCALE,
                )
        # Phase 2 — the recurrent W matmuls, k-accumulating into the slabs.
        if have_w:
            for m in range(KM):
                g, sl = divmod(m, GS)
                for k in range(KM):
                    nc.tensor.matmul(
                        ps_g[g][:, sl, :],
                        wT[:, k, m * 128:(m + 1) * 128],
                        h_prev[:, k, :],
                        start=False,
                        stop=(sl == GS - 1 and k == KM - 1),
                    )
                if sl == GS - 1 and not ABL["no_act"]:
                    nc.scalar.activation(
                        h_new[:, g * GS:(g + 1) * GS, :], ps_g[g][:],
                        mybir.ActivationFunctionType.Tanh, scale=INV_W_SCALE,
                    )
        if ABL["no_act"]:
            # keep a single writer so cross-step deps exist but cost ~0
            nc.scalar.activation(
                h_new[:, 0:1, 0:1], ps_g[-1][:, 0:1, 0:1],
                mybir.ActivationFunctionType.Tanh, scale=INV_W_SCALE,
            )

        # FC contribution of h_{t-1}: PE work overlapping the last tanh
        if FC_MODE == "tail" and t > 0 and not ABL["no_fc"]:
            for k in range(KM):
                if FC_FLIP:
                    nc.tensor.matmul(
                        fc_ps[:], h_prev[:, k, :], wfc_prev[:, k, :],
                        start=(t == 1 and k == 0), stop=False,
                    )
                else:
                    nc.tensor.matmul(
                        fc_ps[:], wfc_prev[:, k, :], h_prev[:, k, :],
                        start=(t == 1 and k == 0), stop=False,
                    )
        h_prev = h_new
        wfc_prev = wfc_t

    for k in range(KM):
        if ABL["no_fc"] and k > 0:
            continue
        a, b = ((h_prev[:, k, :], wfc_prev[:, k, :]) if FC_FLIP
                else (wfc_prev[:, k, :], h_prev[:, k, :]))
        nc.tensor.matmul(
            fc_ps[:], a, b,
            start=(ABL["no_fc"] and k == 0),
            stop=(k == KM - 1 or ABL["no_fc"]),
        )

    out_sb = out_pool.tile([BC, O] if FC_FLIP else [O, BC], F32)
    nc.vector.tensor_copy(out_sb[:], fc_ps[:] if FC_FLIP else fc_ps[:O, :])
    nc.sync.dma_start(out_d[:], out_sb[:])


def _prep_inputs(x, W_i2h, b_i2h, W_h2h, b_h2h, W_fc, t_steps):
    b_total = (np.asarray(b_i2h) + np.asarray(b_h2h)).astype(np.float32)

    # wT[p, kb, c] = W_h2h[c, kb*128+p]  (x W_SCALE, fp8 e3m4 when W_FP8)
    wT = np.ascontiguousarray(
        np.asarray(W_h2h).T.reshape(KM, 128, H).transpose(1, 0, 2)
    ).astype(np.float32) * W_SCALE
    wT = wT.astype(f8e3 if W_FP8 else bf16)

    # wiT[p<64, m, j] = W_i2h[m*128+j, p];  wiT[64, m, j] = b_total[m*128+j]
    # scaled x W_SCALE to match the fp8 W matmuls' PSUM scale
    wiT = np.empty((D + 1, KM, 128), np.float32)
    wiT[:D] = np.asarray(W_i2h).T.reshape(D, KM, 128)
    wiT[D] = b_total.reshape(KM, 128)
    wiT = (wiT * W_SCALE).astype(bf16)

    # wfc[t, p, k, o] = W_fc[o, t*1024 + k*128 + p]
    wfc_core = np.asarray(W_fc)[:, :t_steps * H] \
        .reshape(O, t_steps, KM, 128).transpose(1, 3, 2, 0)
    if FC_FLIP:
        wfc = np.ascontiguousarray(wfc_core).astype(bf16)
    else:
        wfc = np.zeros((t_steps, 128, KM, FC_PAD), bf16)
        wfc[:, :, :, :O] = wfc_core.astype(bf16)

    # per-core xTa[p<64, t, b] = x[c*BC+b, t, p]; xTa[64] = 1.0
    xT = np.asarray(x)[:, :t_steps, :].transpose(2, 1, 0)  # [D, T, B]
    xTas = []
    for c in range(NCORES):
        xa = np.empty((D + 1, t_steps, BC), np.float32)
        xa[:D] = xT[:, :, c * BC:(c + 1) * BC]
        xa[D] = 1.0
        xTas.append(xa.astype(bf16))
    return wT, wiT, wfc, xTas


def _run(x, W_i2h, b_i2h, W_h2h, b_h2h, W_fc, b_fc, t_steps=T, trace=False):
    wT, wiT, wfc, xTas = _prep_inputs(x, W_i2h, b_i2h, W_h2h, b_h2h, W_fc, t_steps)
    nc = _build_program(t_steps)
    in_maps = [
        {"wT": wT, "wiT": wiT, "xTa": xTas[c], "wfc": wfc} for c in range(NCORES)
    ]
    res = run_bass_kernel_spmd(
        nc, in_maps, core_ids=list(range(NCORES)), trace=trace,
        **({"trace_cores": list(range(NCORES))} if trace else {}),
    )
    out = np.empty((B, O), np.float32)
    for c in range(NCORES):
        r = res.results[c]["out"]
        out[c * BC:(c + 1) * BC, :] = r if FC_FLIP else r.T
    out += np.asarray(b_fc, np.float32)[None, :]
    return out, res


def kernel(x, batchSize, W_i2h, b_i2h, W_h2h, b_h2h, W_fc, b_fc):
    out, _ = _run(x, W_i2h, b_i2h, W_h2h, b_h2h, W_fc, b_fc)
    return out

